# revision 58
# baseline (speedup 1.0000x reference)
"""Trainium2 Bass kernel for DotProductAttention (N=8192, D=1024, dk=dv=128).

Returns (out [N,128] f32, attention [N,N] f32) like the reference.

Sequence-parallel over 8 cores with an AllGather K/V exchange:
  - Host: XT = value_in.T; core c receives xt_q = XT[:, c*1024:(c+1)*1024]
    plus the three projection weights (replicated).
  - Device, per core:
      phase 1: K_c first (fp32-native matmuls, exact) so its AllGather can
               launch immediately; then V_c (single-pass f32r on 12-bit hi
               parts) and QT (3-pass compensated f32r, fp32-accurate), all for
               the core's own 1024 rows, accumulated over 8 dmodel chunks in
               PSUM. K and Q are then written as f16 (hi, mid) pairs — 22
               effective mantissa bits.
      phase 2: AllGather the K f16-pair and V f16 across the 8 cores (K's
               reload DMAs issued before the V gather so they don't wait on
               it); V row-chunks transposed f16 straight from the gathered
               DRAM tensor, one DMA-transpose per source core — no SBUF
               staging hop (a staged tile also caused a stack-address-reuse
               WAR that serialized phase 3 behind the V gather).
      phase 3: per 128-row query block: scores = 3-pass f16-pair matmuls
               (error ~3e-4 in score units — softmax at score-std ~1000
               amplifies input rounding, so plain bf16/f16 would fail);
               per-PSUM-block negated row-max on DVE; ACT exp(scale*s + bias)
               in two halves with fused row-sums, written directly to f16;
               DVE normalize (f16 -> f32) and attention written in quarters
               across DMA queues; A@V via one f16 DMA-transpose of the exp'd
               block and 64 accumulated 128x128 f16 matmuls (deferred one
               block so the PE stream never waits on the V gather), scaled by
               1/rowsum.
  - Host: concatenate the per-core blocks.
"""
import numpy as np
import concourse.bass as bass
import concourse.mybir as mybir
import concourse.tile as tile
from concourse.bass_utils import run_bass_kernel_spmd

f32 = mybir.dt.float32
f32r = mybir.dt.float32r
f16 = mybir.dt.float16

N_CORES = 8
N = 8192          # sequence length
D = 1024          # d_model
DK = 128          # d_k == d_v
R = N // N_CORES  # query rows per core (1024)
SCALE = 1.0 / float(np.sqrt(np.float32(DK)))

NB = 512
KC = D // 128     # dmodel chunks (8)


# ---------------------------------------------------------------------------
# walrus in this container accepts at most ONE sync wait per instruction.
# Tile attaches all waits to the consuming instruction; split the extras onto
# same-engine NoOps inserted right before it.
_ctr = [0]


def _split_multiwaits(nc, max_waits=1):
    n_fixed = 0
    for f in nc.m.functions:
        for bb in f.blocks:
            out = []
            changed = False
            for inst in bb.instructions:
                si = inst.sync_info
                if si is not None and si.on_wait and len(si.on_wait) > max_waits:
                    waits = list(si.on_wait)
                    extra, keep = waits[:-max_waits], waits[-max_waits:]
                    for w in extra:
                        _ctr[0] += 1
                        nop = mybir.InstNoOp(
                            name=f"I-mwfix-{_ctr[0]}", ins=[], outs=[])
                        nop.engine = inst.engine
                        nop.sync_info = mybir.SyncInfo(on_wait=[w], on_update=[])
                        out.append(nop)
                    inst.sync_info = mybir.SyncInfo(
                        on_wait=keep, on_update=list(si.on_update))
                    n_fixed += 1
                    changed = True
                out.append(inst)
            if changed:
                bb.instructions = out
    return n_fixed


# ---------------------------------------------------------------------------
def build_nc():
    nc = bass.Bass("TRN2", target_bir_lowering=False)

    xt_q = nc.dram_tensor("xt_q", [D, R], f32, kind="ExternalInput")
    wq_d = nc.dram_tensor("wq", [D, DK], f32, kind="ExternalInput")
    wk_d = nc.dram_tensor("wk", [D, DK], f32, kind="ExternalInput")
    wv_d = nc.dram_tensor("wv", [D, DK], f32, kind="ExternalInput")
    attn_d = nc.dram_tensor("attn", [R, N], f32, kind="ExternalOutput")
    out_d = nc.dram_tensor("out", [R, DK], f32, kind="ExternalOutput")

    # internal DRAM for the K/V exchange; K travels as an f16 (hi, mid) pair
    # (22-bit effective mantissa — plenty: score error stays ~3e-4)
    kt_part = nc.dram_tensor("kt_part", [2, DK, R], f16)
    vt_part = nc.dram_tensor("vt_part", [DK, R], f16)
    kt_all = nc.dram_tensor("kt_all", [N_CORES, 2, DK, R], f16, addr_space="Shared")
    vt_all = nc.dram_tensor("vt_all", [N_CORES, DK, R], f16, addr_space="Shared")

    n_qb = R // 128           # 8 query blocks per core
    n_kc = N // 128           # 64 k-chunks

    with tile.TileContext(nc) as tc:
        with tc.tile_pool(name="persist", bufs=1) as persist:
            kt_hi = persist.tile([128, N], f16, tag="kt_hi")
            kt_mid = persist.tile([128, N], f16, tag="kt_mid")
            qt_hi = persist.tile([128, R], f16, tag="qt_hi")
            qt_mid = persist.tile([128, R], f16, tag="qt_mid")
            v_sb = persist.tile([128, n_kc, 128], f16, tag="v_sb")

            # ---- phase 1: projections for own rows ----------------------
            with tc.tile_pool(name="pw", bufs=1) as pw, \
                 tc.tile_pool(name="p1", bufs=2) as p1, \
                 tc.tile_pool(name="p1x", bufs=1) as p1x, \
                 tc.tile_pool(name="ps1", bufs=1, space="PSUM") as ps1:
                wq_t = pw.tile([128, KC, DK], f32, tag="wq_t")
                wk_t = pw.tile([128, KC, DK], f32, tag="wk_t")
                wv_t = pw.tile([128, KC, DK], f32, tag="wv_t")
                nc.sync.dma_start(
                    out=wk_t[:, 0:2, :],
                    in_=wk_d[0:256, :].rearrange("(c k) d -> k c d", k=128))
                nc.sync.dma_start(
                    out=wk_t[:, 2:KC, :],
                    in_=wk_d[256:D, :].rearrange("(c k) d -> k c d", k=128))
                for wd, wt in ((wq_d, wq_t), (wv_d, wv_t)):
                    nc.sync.dma_start(
                        out=wt, in_=wd.rearrange("(c k) d -> k c d", k=128))
                wq_hi = pw.tile([128, KC, DK], f32r, tag="wq_hi")
                wq_lo = pw.tile([128, KC, DK], f32r, tag="wq_lo")
                wv_hi = pw.tile([128, KC, DK], f32r, tag="wv_hi")
                nc.vector.tensor_copy(out=wq_hi, in_=wq_t)
                nc.vector.tensor_tensor(
                    out=wq_lo, in0=wq_t, in1=wq_hi.bitcast(f32),
                    op=mybir.AluOpType.subtract)
                nc.vector.tensor_copy(out=wv_hi, in_=wv_t)

                kt_ps = ps1.tile([128, R], f32, tag="kt_ps")
                vt_ps = ps1.tile([128, R], f32, tag="vt_ps")
                qt_ps = ps1.tile([128, R], f32, tag="qt_ps")
                # pass 1: K only, so its AllGather can launch ASAP
                xtq_tiles = []
                for c in range(KC):
                    xtq_c = p1x.tile([128, R], f32, tag=f"xtq_c{c}")
                    for dh in range(4):
                        nc.sync.dma_start(
                            out=xtq_c[:, bass.ds(dh * (R // 4), R // 4)],
                            in_=xt_q[c * 128:(c + 1) * 128,
                                     dh * (R // 4):(dh + 1) * (R // 4)])
                    xtq_tiles.append(xtq_c)
                    first, last = c == 0, c == KC - 1
                    for h in range(R // NB):
                        sl = bass.ds(h * NB, NB)
                        nc.tensor.matmul(kt_ps[:, sl], wk_t[:, c, :], xtq_c[:, sl],
                                         start=first, stop=last)
                kt_ev_hi = p1.tile([128, R], f16, tag="kt_ev_hi")
                kt_ev_mid = p1.tile([128, R], f16, tag="kt_ev_mid")
                nc.vector.tensor_copy(out=kt_ev_hi, in_=kt_ps)
                nc.vector.tensor_tensor(
                    out=kt_ev_mid, in0=kt_ps, in1=kt_ev_hi,
                    op=mybir.AluOpType.subtract)
                nc.sync.dma_start(out=kt_part[0, :, :], in_=kt_ev_hi)
                nc.sync.dma_start(out=kt_part[1, :, :], in_=kt_ev_mid)
                nc.gpsimd.collective_compute(
                    "AllGather", mybir.AluOpType.bypass,
                    replica_groups=[list(range(N_CORES))],
                    ins=[kt_part[:, :, :]], outs=[kt_all[:, :, :, :]])
                # K pair reload, emitted right after the K gather so its
                # dependency clock does not cover the V gather
                for c in range(N_CORES):
                    sl = bass.ds(c * R, R)
                    nc.sync.dma_start(out=kt_hi[:, sl], in_=kt_all[c, 0, :, :])
                    nc.sync.dma_start(out=kt_mid[:, sl], in_=kt_all[c, 1, :, :])

                # pass 2: V then Q (overlaps the K gather)
                for c in range(KC):
                    xtq_c = xtq_tiles[c]
                    xtq_hi = p1.tile([128, R], f32r, tag="xtq_hi")
                    xtq_lo = p1.tile([128, R], f32r, tag="xtq_lo")
                    nc.vector.tensor_copy(out=xtq_hi, in_=xtq_c)
                    nc.vector.tensor_tensor(
                        out=xtq_lo, in0=xtq_c, in1=xtq_hi.bitcast(f32),
                        op=mybir.AluOpType.subtract)
                    first, last = c == 0, c == KC - 1
                    for h in range(R // NB):
                        sl = bass.ds(h * NB, NB)
                        nc.tensor.matmul(vt_ps[:, sl], wv_hi[:, c, :], xtq_hi[:, sl],
                                         start=first, stop=last)
                        nc.tensor.matmul(qt_ps[:, sl], wq_hi[:, c, :], xtq_hi[:, sl],
                                         start=first, stop=False)
                        nc.tensor.matmul(qt_ps[:, sl], wq_hi[:, c, :], xtq_lo[:, sl],
                                         start=False, stop=False)
                        nc.tensor.matmul(qt_ps[:, sl], wq_lo[:, c, :], xtq_hi[:, sl],
                                         start=False, stop=last)

                vt_ev = p1.tile([128, R], f16, tag="vt_ev")
                nc.scalar.copy(out=vt_ev, in_=vt_ps)
                nc.sync.dma_start(out=vt_part[:, :], in_=vt_ev)
                nc.gpsimd.collective_compute(
                    "AllGather", mybir.AluOpType.bypass,
                    replica_groups=[list(range(N_CORES))],
                    ins=[vt_part[:, :]], outs=[vt_all[:, :, :]])
                nc.vector.tensor_copy(out=qt_hi, in_=qt_ps)
                nc.vector.tensor_tensor(
                    out=qt_mid, in0=qt_ps, in1=qt_hi,
                    op=mybir.AluOpType.subtract)

            # V chunks via per-source-core DMA transposes straight from the
            # gathered DRAM tensor (no SBUF staging hop)
            for c in range(N_CORES):
                nc.sync.dma_start_transpose(
                    out=v_sb[:, bass.ds(c * (n_kc // N_CORES), n_kc // N_CORES), :],
                    in_=vt_all[c, :, :])

            # ---- phase 3: attention per query block ---------------------
            with tc.tile_pool(name="p3", bufs=2) as p3, \
                 tc.tile_pool(name="p3a", bufs=2) as p3a, \
                 tc.tile_pool(name="p3t", bufs=2) as p3t, \
                 tc.tile_pool(name="stats", bufs=3) as stats, \
                 tc.tile_pool(name="ps_s", bufs=3, space="PSUM") as ps_s, \
                 tc.tile_pool(name="ps_o", bufs=2, space="PSUM") as ps_o:
                def emit_pv(qb, pt_all, recip):
                    # A@V for block qb (deferred one iteration so the PE
                    # stream never stalls waiting for the gathered V)
                    ops = ps_o.tile([128, DK], f32, tag="ops")
                    for c in range(n_kc):
                        nc.tensor.matmul(
                            ops, pt_all[:, c, :], v_sb[:, c, :],
                            start=(c == 0), stop=(c == n_kc - 1))
                    o_sb = stats.tile([128, DK], f32, tag="o_sb")
                    nc.vector.tensor_scalar_mul(out=o_sb, in0=ops, scalar1=recip)
                    nc.sync.dma_start(
                        out=out_d[qb * 128:(qb + 1) * 128, :], in_=o_sb)

                pending = []
                for qb in range(n_qb):
                    qsl = bass.ts(qb, 128)
                    s_tile = p3.tile([128, N], f32, tag="s_tile")
                    negmax_blk = stats.tile([128, 8], f32, tag="negmax_blk")
                    a16 = p3a.tile([128, N], f16, tag="a16")
                    sumexp = stats.tile([128, 2], f32, tag="sumexp")
                    recip = stats.tile([128, 1], f32, tag="recip")
                    bias_t = stats.tile([128, 1], f32, tag="bias")
                    for half in range(2):
                        for hbl in range(4):
                            hb = half * 4 + hbl
                            sps = ps_s.tile([128, 1024], f32, tag="sps")
                            for h in range(2):
                                ks = bass.ds(hb * 1024 + h * NB, NB)
                                po = sps[:, bass.ds(h * NB, NB)]
                                nc.tensor.matmul(po, qt_hi[:, qsl], kt_hi[:, ks],
                                                 start=True, stop=False)
                                nc.tensor.matmul(po, qt_hi[:, qsl], kt_mid[:, ks],
                                                 start=False, stop=False)
                                nc.tensor.matmul(po, qt_mid[:, qsl], kt_hi[:, ks],
                                                 start=False, stop=True)
                            dst = s_tile[:, bass.ds(hb * 1024, 1024)]
                            # block max (negated) straight off PSUM, pipelined
                            nc.vector.tensor_reduce(
                                out=negmax_blk[:, hb:hb + 1], in_=sps,
                                axis=mybir.AxisListType.X, op=mybir.AluOpType.max,
                                negate=True)
                            nc.scalar.copy(out=dst, in_=sps)
                        if half == 0:
                            continue
                        # bias = SCALE * (-rowmax), from the 8 block maxes
                        nc.vector.tensor_reduce(
                            out=bias_t, in_=negmax_blk, axis=mybir.AxisListType.X,
                            op=mybir.AluOpType.min)
                        nc.vector.tensor_scalar_mul(
                            out=bias_t, in0=bias_t, scalar1=SCALE)
                        # exp in halves; f16 out feeds the A@V transpose
                        for eq in range(2):
                            esl = bass.ds(eq * (N // 2), N // 2)
                            nc.scalar.activation(
                                out=a16[:, esl], in_=s_tile[:, esl],
                                func=mybir.ActivationFunctionType.Exp,
                                bias=bias_t, scale=SCALE,
                                accum_out=sumexp[:, eq:eq + 1])
                        nc.vector.tensor_reduce(
                            out=recip, in_=sumexp, axis=mybir.AxisListType.X,
                            op=mybir.AluOpType.add)
                        nc.vector.reciprocal(out=recip, in_=recip)
                        # normalize f16 -> f32 and write attention in quarters
                        # across DMA queues
                        for eq in range(4):
                            off = eq * (N // 4)
                            esl = bass.ds(off, N // 4)
                            nc.vector.tensor_scalar_mul(
                                out=s_tile[:, esl], in0=a16[:, esl], scalar1=recip)
                            nc.sync.dma_start(
                                out=attn_d[qb * 128:(qb + 1) * 128,
                                           off:off + N // 4],
                                in_=s_tile[:, esl])

                    if len(pending) >= 1:
                        emit_pv(*pending.pop(0))
                    pt_all = p3t.tile([128, n_kc, 128], f16, tag="pt_all")
                    nc.sync.dma_start_transpose(out=pt_all, in_=a16)
                    pending.append((qb, pt_all, recip))
                for args in pending:
                    emit_pv(*args)

    _split_multiwaits(nc)
    return nc


_NC_CACHE = None


def _in_maps(value_in, w_query, w_key, w_value):
    xt = np.ascontiguousarray(value_in.T)            # [D, N]
    maps = []
    for c in range(N_CORES):
        maps.append({
            "xt_q": np.ascontiguousarray(xt[:, c * R:(c + 1) * R]),
            "wq": w_query,
            "wk": w_key,
            "wv": w_value,
        })
    return maps


def kernel(value_in, w_query, w_key, w_value):
    global _NC_CACHE
    value_in = np.ascontiguousarray(np.asarray(value_in, dtype=np.float32))
    w_query = np.ascontiguousarray(np.asarray(w_query, dtype=np.float32))
    w_key = np.ascontiguousarray(np.asarray(w_key, dtype=np.float32))
    w_value = np.ascontiguousarray(np.asarray(w_value, dtype=np.float32))

    if _NC_CACHE is None:
        _NC_CACHE = build_nc()
    nc = _NC_CACHE

    res = run_bass_kernel_spmd(
        nc, _in_maps(value_in, w_query, w_key, w_value),
        core_ids=list(range(N_CORES)))
    attention = np.concatenate([r["attn"] for r in res.results], axis=0)
    out = np.concatenate([r["out"] for r in res.results], axis=0)
    return out, attention


# revision 59
# speedup vs baseline: 1.0067x; 1.0067x over previous
"""Trainium2 Bass kernel for DotProductAttention (N=8192, D=1024, dk=dv=128).

Returns (out [N,128] f32, attention [N,N] f32) like the reference.

Sequence-parallel over 8 cores with an AllGather K/V exchange:
  - Host: XT = value_in.T; core c receives xt_q = XT[:, c*1024:(c+1)*1024]
    plus the three projection weights (replicated).
  - Device, per core:
      phase 1: K_c first (fp32-native matmuls, exact) so its AllGather can
               launch immediately; then V_c (single-pass f32r on 12-bit hi
               parts) and QT (3-pass compensated f32r, fp32-accurate), all for
               the core's own 1024 rows, accumulated over 8 dmodel chunks in
               PSUM. K and Q are then written as f16 (hi, mid) pairs — 22
               effective mantissa bits.
      phase 2: AllGather the K f16-pair and V f16 across the 8 cores (K's
               reload DMAs issued before the V gather so they don't wait on
               it); V row-chunks transposed f16 straight from the gathered
               DRAM tensor, one DMA-transpose per source core — no SBUF
               staging hop (a staged tile also caused a stack-address-reuse
               WAR that serialized phase 3 behind the V gather).
      phase 3: per 128-row query block: scores = 3-pass f16-pair matmuls
               (error ~3e-4 in score units — softmax at score-std ~1000
               amplifies input rounding, so plain bf16/f16 would fail);
               per-PSUM-block negated row-max on DVE; ACT exp(scale*s + bias)
               in two halves with fused row-sums, written directly to f16;
               DVE normalize (f16 -> f32) and attention written in quarters
               across DMA queues; A@V via one f16 DMA-transpose of the exp'd
               block and 64 accumulated 128x128 f16 matmuls (deferred one
               block so the PE stream never waits on the V gather), scaled by
               1/rowsum.
  - Host: concatenate the per-core blocks.
"""
import numpy as np
import concourse.bass as bass
import concourse.mybir as mybir
import concourse.tile as tile
from concourse.bass_utils import run_bass_kernel_spmd

f32 = mybir.dt.float32
f32r = mybir.dt.float32r
f16 = mybir.dt.float16

N_CORES = 8
N = 8192          # sequence length
D = 1024          # d_model
DK = 128          # d_k == d_v
R = N // N_CORES  # query rows per core (1024)
SCALE = 1.0 / float(np.sqrt(np.float32(DK)))

NB = 512
KC = D // 128     # dmodel chunks (8)


# ---------------------------------------------------------------------------
# walrus in this container accepts at most ONE sync wait per instruction.
# Tile attaches all waits to the consuming instruction; split the extras onto
# same-engine NoOps inserted right before it.
_ctr = [0]


def _split_multiwaits(nc, max_waits=1):
    n_fixed = 0
    for f in nc.m.functions:
        for bb in f.blocks:
            out = []
            changed = False
            for inst in bb.instructions:
                si = inst.sync_info
                if si is not None and si.on_wait and len(si.on_wait) > max_waits:
                    waits = list(si.on_wait)
                    extra, keep = waits[:-max_waits], waits[-max_waits:]
                    for w in extra:
                        _ctr[0] += 1
                        nop = mybir.InstNoOp(
                            name=f"I-mwfix-{_ctr[0]}", ins=[], outs=[])
                        nop.engine = inst.engine
                        nop.sync_info = mybir.SyncInfo(on_wait=[w], on_update=[])
                        out.append(nop)
                    inst.sync_info = mybir.SyncInfo(
                        on_wait=keep, on_update=list(si.on_update))
                    n_fixed += 1
                    changed = True
                out.append(inst)
            if changed:
                bb.instructions = out
    return n_fixed


# ---------------------------------------------------------------------------
def build_nc():
    nc = bass.Bass("TRN2", target_bir_lowering=False)

    xt_q = nc.dram_tensor("xt_q", [D, R], f32, kind="ExternalInput")
    wq_d = nc.dram_tensor("wq", [D, DK], f32, kind="ExternalInput")
    wk_d = nc.dram_tensor("wk", [D, DK], f32, kind="ExternalInput")
    wv_d = nc.dram_tensor("wv", [D, DK], f32, kind="ExternalInput")
    attn_d = nc.dram_tensor("attn", [R, N], f32, kind="ExternalOutput")
    out_d = nc.dram_tensor("out", [R, DK], f32, kind="ExternalOutput")

    # internal DRAM for the K/V exchange; K travels as an f16 (hi, mid) pair
    # (22-bit effective mantissa — plenty: score error stays ~3e-4)
    kt_part = nc.dram_tensor("kt_part", [2, DK, R], f16)
    vt_part = nc.dram_tensor("vt_part", [DK, R], f16)
    kt_all = nc.dram_tensor("kt_all", [N_CORES, 2, DK, R], f16, addr_space="Shared")
    vt_all = nc.dram_tensor("vt_all", [N_CORES, DK, R], f16, addr_space="Shared")

    n_qb = R // 128           # 8 query blocks per core
    n_kc = N // 128           # 64 k-chunks

    with tile.TileContext(nc) as tc:
        with tc.tile_pool(name="persist", bufs=1) as persist:
            kt_hi = persist.tile([128, N], f16, tag="kt_hi")
            kt_mid = persist.tile([128, N], f16, tag="kt_mid")
            qt_hi = persist.tile([128, R], f16, tag="qt_hi")
            qt_mid = persist.tile([128, R], f16, tag="qt_mid")
            v_sb = persist.tile([128, n_kc, 128], f16, tag="v_sb")

            # ---- phase 1: projections for own rows ----------------------
            with tc.tile_pool(name="pw", bufs=1) as pw, \
                 tc.tile_pool(name="p1", bufs=2) as p1, \
                 tc.tile_pool(name="p1x", bufs=1) as p1x, \
                 tc.tile_pool(name="ps1", bufs=1, space="PSUM") as ps1:
                wq_t = pw.tile([128, KC, DK], f32, tag="wq_t")
                wk_t = pw.tile([128, KC, DK], f32, tag="wk_t")
                wv_t = pw.tile([128, KC, DK], f32, tag="wv_t")
                nc.sync.dma_start(
                    out=wk_t[:, 0:2, :],
                    in_=wk_d[0:256, :].rearrange("(c k) d -> k c d", k=128))
                nc.sync.dma_start(
                    out=wk_t[:, 2:KC, :],
                    in_=wk_d[256:D, :].rearrange("(c k) d -> k c d", k=128))
                for wd, wt in ((wq_d, wq_t), (wv_d, wv_t)):
                    nc.sync.dma_start(
                        out=wt, in_=wd.rearrange("(c k) d -> k c d", k=128))
                wq_hi = pw.tile([128, KC, DK], f32r, tag="wq_hi")
                wq_lo = pw.tile([128, KC, DK], f32r, tag="wq_lo")
                wv_hi = pw.tile([128, KC, DK], f32r, tag="wv_hi")
                nc.vector.tensor_copy(out=wq_hi, in_=wq_t)
                nc.vector.tensor_tensor(
                    out=wq_lo, in0=wq_t, in1=wq_hi.bitcast(f32),
                    op=mybir.AluOpType.subtract)
                nc.vector.tensor_copy(out=wv_hi, in_=wv_t)

                kt_ps = ps1.tile([128, R], f32, tag="kt_ps")
                vt_ps = ps1.tile([128, R], f32, tag="vt_ps")
                qt_ps = ps1.tile([128, R], f32, tag="qt_ps")
                # pass 1: K only, so its AllGather can launch ASAP
                xtq_tiles = []
                for c in range(KC):
                    xtq_c = p1x.tile([128, R], f32, tag=f"xtq_c{c}")
                    for dh in range(4):
                        nc.sync.dma_start(
                            out=xtq_c[:, bass.ds(dh * (R // 4), R // 4)],
                            in_=xt_q[c * 128:(c + 1) * 128,
                                     dh * (R // 4):(dh + 1) * (R // 4)])
                    xtq_tiles.append(xtq_c)
                    first, last = c == 0, c == KC - 1
                    for h in range(R // NB):
                        sl = bass.ds(h * NB, NB)
                        nc.tensor.matmul(kt_ps[:, sl], wk_t[:, c, :], xtq_c[:, sl],
                                         start=first, stop=last)
                kt_ev_hi = p1.tile([128, R], f16, tag="kt_ev_hi")
                kt_ev_mid = p1.tile([128, R], f16, tag="kt_ev_mid")
                nc.vector.tensor_copy(out=kt_ev_hi, in_=kt_ps)
                nc.vector.tensor_tensor(
                    out=kt_ev_mid, in0=kt_ps, in1=kt_ev_hi,
                    op=mybir.AluOpType.subtract)
                nc.sync.dma_start(out=kt_part[0, :, :], in_=kt_ev_hi)
                nc.sync.dma_start(out=kt_part[1, :, :], in_=kt_ev_mid)
                nc.gpsimd.collective_compute(
                    "AllGather", mybir.AluOpType.bypass,
                    replica_groups=[list(range(N_CORES))],
                    ins=[kt_part[:, :, :]], outs=[kt_all[:, :, :, :]])
                # K pair reload, emitted right after the K gather so its
                # dependency clock does not cover the V gather
                for c in range(N_CORES):
                    sl = bass.ds(c * R, R)
                    nc.sync.dma_start(out=kt_hi[:, sl], in_=kt_all[c, 0, :, :])
                    nc.sync.dma_start(out=kt_mid[:, sl], in_=kt_all[c, 1, :, :])

                # pass 2: V then Q (overlaps the K gather)
                for c in range(KC):
                    xtq_c = xtq_tiles[c]
                    xtq_hi = p1.tile([128, R], f32r, tag="xtq_hi")
                    xtq_lo = p1.tile([128, R], f32r, tag="xtq_lo")
                    nc.vector.tensor_copy(out=xtq_hi, in_=xtq_c)
                    nc.vector.tensor_tensor(
                        out=xtq_lo, in0=xtq_c, in1=xtq_hi.bitcast(f32),
                        op=mybir.AluOpType.subtract)
                    first, last = c == 0, c == KC - 1
                    for h in range(R // NB):
                        sl = bass.ds(h * NB, NB)
                        nc.tensor.matmul(vt_ps[:, sl], wv_hi[:, c, :], xtq_hi[:, sl],
                                         start=first, stop=last)
                        nc.tensor.matmul(qt_ps[:, sl], wq_hi[:, c, :], xtq_hi[:, sl],
                                         start=first, stop=False)
                        nc.tensor.matmul(qt_ps[:, sl], wq_hi[:, c, :], xtq_lo[:, sl],
                                         start=False, stop=False)
                        nc.tensor.matmul(qt_ps[:, sl], wq_lo[:, c, :], xtq_hi[:, sl],
                                         start=False, stop=last)

                vt_ev = p1.tile([128, R], f16, tag="vt_ev")
                nc.scalar.copy(out=vt_ev, in_=vt_ps)
                nc.sync.dma_start(out=vt_part[:, :], in_=vt_ev)
                nc.gpsimd.collective_compute(
                    "AllGather", mybir.AluOpType.bypass,
                    replica_groups=[list(range(N_CORES))],
                    ins=[vt_part[:, :]], outs=[vt_all[:, :, :]])
                nc.vector.tensor_copy(out=qt_hi, in_=qt_ps)
                nc.vector.tensor_tensor(
                    out=qt_mid, in0=qt_ps, in1=qt_hi,
                    op=mybir.AluOpType.subtract)

            # V chunks via per-source-core DMA transposes straight from the
            # gathered DRAM tensor (no SBUF staging hop)
            for c in range(N_CORES):
                nc.sync.dma_start_transpose(
                    out=v_sb[:, bass.ds(c * (n_kc // N_CORES), n_kc // N_CORES), :],
                    in_=vt_all[c, :, :])

            # ---- phase 3: attention per query block ---------------------
            with tc.tile_pool(name="p3", bufs=2) as p3, \
                 tc.tile_pool(name="p3a", bufs=2) as p3a, \
                 tc.tile_pool(name="p3t", bufs=2) as p3t, \
                 tc.tile_pool(name="stats", bufs=3) as stats, \
                 tc.tile_pool(name="ps_s", bufs=6, space="PSUM") as ps_s, \
                 tc.tile_pool(name="ps_o", bufs=2, space="PSUM") as ps_o:
                def emit_pv(qb, pt_all, recip):
                    # A@V for block qb (deferred one iteration so the PE
                    # stream never stalls waiting for the gathered V)
                    ops = ps_o.tile([128, DK], f32, tag="ops")
                    for c in range(n_kc):
                        nc.tensor.matmul(
                            ops, pt_all[:, c, :], v_sb[:, c, :],
                            start=(c == 0), stop=(c == n_kc - 1))
                    o_sb = stats.tile([128, DK], f32, tag="o_sb")
                    nc.vector.tensor_scalar_mul(out=o_sb, in0=ops, scalar1=recip)
                    nc.sync.dma_start(
                        out=out_d[qb * 128:(qb + 1) * 128, :], in_=o_sb)

                pending = []
                for qb in range(n_qb):
                    qsl = bass.ts(qb, 128)
                    s_tile = p3.tile([128, N], f32, tag="s_tile")
                    negmax_blk = stats.tile([128, 16], f32, tag="negmax_blk")
                    a16 = p3a.tile([128, N], f16, tag="a16")
                    sumexp = stats.tile([128, 2], f32, tag="sumexp")
                    recip = stats.tile([128, 1], f32, tag="recip")
                    bias_t = stats.tile([128, 1], f32, tag="bias")
                    for half in range(2):
                        for hbl in range(8):
                            hb = half * 8 + hbl
                            sps = ps_s.tile([128, NB], f32, tag="sps")
                            ks = bass.ds(hb * NB, NB)
                            nc.tensor.matmul(sps, qt_hi[:, qsl], kt_hi[:, ks],
                                             start=True, stop=False)
                            nc.tensor.matmul(sps, qt_hi[:, qsl], kt_mid[:, ks],
                                             start=False, stop=False)
                            nc.tensor.matmul(sps, qt_mid[:, qsl], kt_hi[:, ks],
                                             start=False, stop=True)
                            dst = s_tile[:, bass.ds(hb * NB, NB)]
                            # block max (negated) straight off PSUM, pipelined
                            nc.vector.tensor_reduce(
                                out=negmax_blk[:, hb:hb + 1], in_=sps,
                                axis=mybir.AxisListType.X, op=mybir.AluOpType.max,
                                negate=True)
                            nc.scalar.copy(out=dst, in_=sps)
                        if half == 0:
                            continue
                        # bias = SCALE * (-rowmax), from the 8 block maxes
                        nc.vector.tensor_reduce(
                            out=bias_t, in_=negmax_blk, axis=mybir.AxisListType.X,
                            op=mybir.AluOpType.min)
                        nc.vector.tensor_scalar_mul(
                            out=bias_t, in0=bias_t, scalar1=SCALE)
                        # exp in halves; f16 out feeds the A@V transpose
                        for eq in range(2):
                            esl = bass.ds(eq * (N // 2), N // 2)
                            nc.scalar.activation(
                                out=a16[:, esl], in_=s_tile[:, esl],
                                func=mybir.ActivationFunctionType.Exp,
                                bias=bias_t, scale=SCALE,
                                accum_out=sumexp[:, eq:eq + 1])
                        nc.vector.tensor_reduce(
                            out=recip, in_=sumexp, axis=mybir.AxisListType.X,
                            op=mybir.AluOpType.add)
                        nc.vector.reciprocal(out=recip, in_=recip)
                        # normalize f16 -> f32 and write attention in quarters
                        # across DMA queues
                        for eq in range(4):
                            off = eq * (N // 4)
                            esl = bass.ds(off, N // 4)
                            nc.vector.tensor_scalar_mul(
                                out=s_tile[:, esl], in0=a16[:, esl], scalar1=recip)
                            nc.sync.dma_start(
                                out=attn_d[qb * 128:(qb + 1) * 128,
                                           off:off + N // 4],
                                in_=s_tile[:, esl])

                    if len(pending) >= 1:
                        emit_pv(*pending.pop(0))
                    pt_all = p3t.tile([128, n_kc, 128], f16, tag="pt_all")
                    nc.sync.dma_start_transpose(out=pt_all, in_=a16)
                    pending.append((qb, pt_all, recip))
                for args in pending:
                    emit_pv(*args)

    _split_multiwaits(nc)
    return nc


_NC_CACHE = None


def _in_maps(value_in, w_query, w_key, w_value):
    xt = np.ascontiguousarray(value_in.T)            # [D, N]
    maps = []
    for c in range(N_CORES):
        maps.append({
            "xt_q": np.ascontiguousarray(xt[:, c * R:(c + 1) * R]),
            "wq": w_query,
            "wk": w_key,
            "wv": w_value,
        })
    return maps


def kernel(value_in, w_query, w_key, w_value):
    global _NC_CACHE
    value_in = np.ascontiguousarray(np.asarray(value_in, dtype=np.float32))
    w_query = np.ascontiguousarray(np.asarray(w_query, dtype=np.float32))
    w_key = np.ascontiguousarray(np.asarray(w_key, dtype=np.float32))
    w_value = np.ascontiguousarray(np.asarray(w_value, dtype=np.float32))

    if _NC_CACHE is None:
        _NC_CACHE = build_nc()
    nc = _NC_CACHE

    res = run_bass_kernel_spmd(
        nc, _in_maps(value_in, w_query, w_key, w_value),
        core_ids=list(range(N_CORES)))
    attention = np.concatenate([r["attn"] for r in res.results], axis=0)
    out = np.concatenate([r["out"] for r in res.results], axis=0)
    return out, attention
